# revision 39
# baseline (speedup 1.0000x reference)
"""Trainium2 Bass kernel for nn_BASE_MAMBA_14018773254552.

Mamba block (d_model=128, d_inner=256, d_state=64, d_conv=4, L=1024, B=4)
+ input proj + classifier head.

Sharding: 8 cores = 4 batches x 2 d_inner-halves (128 channels each).
Each core computes its batch's full front-end (input proj, in_proj, conv,
x_proj) feature-major ([feature, time] tiles), then the selective scan for
its 128-channel half, and the partial out-proj + mean-pool. The host sums
the two channel-half partials per batch and runs the tiny classifier
(BatchNorm couples batches, so it cannot live on one core).

Scan-loop design (DVE-bound; see session notes):
- pair p = channels (2p, 2p+1); scan tile partitions hold (state n, j).
- pairs are merged into groups of up to 4: ONE tensor_tensor_scan over
  [128, 4096] replaces 4 scans -- exact because dA's first column of
  each inner pair is zeroed (h = 0*h_prev + dBx resets the recurrence).
  Group sizes taper (1,1,2,4...4,2,2,2,1,1) so the first scan starts
  before full warmup and the last group's serial exit chain is short.
- dBx/HC multiplies run merged and in-place (urep/Ht buffers).
- dt replication runs on PE via one-hot selR matmuls into PSUM (a DMA
  replication of dt measured slower: SBUF write-port contention slows
  every DVE op by ~20%).
- u replication via DMA from a DRAM bounce, alternating the two HWDGE
  queues (sync + scalar-triggered); B/C replication via gpsimd SWDGE.
- ACT tables reload on every function switch (1.28us), so table-using
  activations are batched by function: Silu x3, Sigmoid, Ln, then Exp
  for all dA tiles.

Self-contained: hardcodes all shapes; builds + compiles the Bass program
once per process and runs it on cores 0-7 via run_bass_kernel_spmd.
"""
import numpy as np

try:
    import concourse.bacc as bacc
except ImportError:  # pragma: no cover - path fallback
    import sys
    for _p in ("/opt/trn_rl_repo", "/root/.axon_site/_ro/trn_rl_repo"):
        if _p not in sys.path:
            sys.path.insert(0, _p)
    import concourse.bacc as bacc

import ml_dtypes
import concourse.bass as bass
import concourse.mybir as mybir
import concourse.tile as tile
from concourse.bass_utils import run_bass_kernel_spmd

F32 = mybir.dt.float32
BF16 = mybir.dt.bfloat16
AF = mybir.ActivationFunctionType
OP = mybir.AluOpType

B, L, CIN = 4, 1024, 20
DM, DS, DC = 128, 64, 4
DI = 256
DTR = 8
DH = 128          # channels per core (d_inner half)
EPS = 1e-5

_cache = {}


def _build():
    nc = bacc.Bacc("TRN2", target_bir_lowering=False, debug=False, num_devices=8)

    # ---- I/O ----
    xt_d = nc.dram_tensor("xt", [CIN, L], F32, kind="ExternalInput")
    wpT_d = nc.dram_tensor("wpT", [CIN, DM], F32, kind="ExternalInput")
    bp_d = nc.dram_tensor("bp", [DM, 1], F32, kind="ExternalInput")
    wiT_d = nc.dram_tensor("wiT", [DM, 3 * DH], F32, kind="ExternalInput")
    convw_d = nc.dram_tensor("convw", [DH, 2 * DC], F32, kind="ExternalInput")
    convb_d = nc.dram_tensor("convb", [DH, 2], F32, kind="ExternalInput")
    wxT_d = nc.dram_tensor("wxT", [DH, 2 * 136], F32, kind="ExternalInput")
    wdtT_d = nc.dram_tensor("wdtT", [DTR, DH], F32, kind="ExternalInput")
    bdt_d = nc.dram_tensor("bdt", [DH, 1], F32, kind="ExternalInput")
    alogp_d = nc.dram_tensor("alogp", [DH, DS], F32, kind="ExternalInput")
    dskip_d = nc.dram_tensor("dskip", [DH, 1], F32, kind="ExternalInput")
    woutT_d = nc.dram_tensor("woutT", [DH, DM], F32, kind="ExternalInput")
    selE_d = nc.dram_tensor("selE", [DH, DS * DH], BF16, kind="ExternalInput")
    selR_d = nc.dram_tensor("selR", [DH, DS * DH], BF16, kind="ExternalInput")
    pooled_d = nc.dram_tensor("pooled", [DM, 1], F32, kind="ExternalOutput")
    u_scr = nc.dram_tensor("u_scr", [DH, L], BF16)
    dt_scr = nc.dram_tensor("dt_scr", [DH, L], BF16)
    bm_scr = nc.dram_tensor("bm_scr", [DS, L], BF16)
    cm_scr = nc.dram_tensor("cm_scr", [DS, L], BF16)

    with tile.TileContext(nc) as tc:
        with (
            tc.tile_pool(name="const", bufs=1) as cp,
            tc.tile_pool(name="work", bufs=1) as wp,
        ):
            # ---- load params ----
            xt = cp.tile([CIN, L], F32)
            wpT = cp.tile([CIN, DM], F32)
            bp = cp.tile([DM, 1], F32)
            wiT = cp.tile([DM, 3 * DH], F32)
            convw = cp.tile([DH, 2 * DC], F32)
            convb = cp.tile([DH, 2], F32)
            wxT = cp.tile([DH, 2 * 136], F32)
            wdtT = cp.tile([DTR, DH], F32)
            bdt = cp.tile([DH, 1], F32)
            alogp = cp.tile([DH, DS], F32)
            dskip = cp.tile([DH, 1], F32)
            woutT = cp.tile([DH, DM], F32)
            selE = cp.tile([DH, DS * DH], BF16)
            selR = cp.tile([DH, DS * DH], BF16)
            for t_, d_ in [(xt, xt_d), (wpT, wpT_d), (bp, bp_d), (wiT, wiT_d),
                           (convw, convw_d), (convb, convb_d), (wxT, wxT_d),
                           (wdtT, wdtT_d), (bdt, bdt_d), (alogp, alogp_d),
                           (dskip, dskip_d), (woutT, woutT_d), (selE, selE_d),
                           (selR, selR_d)]:
                nc.sync.dma_start(t_[:], d_[:])

            HLF = (slice(0, 512), slice(512, 1024))

            # ---- phase 1: front-end ----
            # ACT tables reload on every function switch (1.28us) so all
            # table-using ACTs run full-length, batched by function; the
            # DVE/PE/copy stages pipeline in time-halves.
            bdtn = wp.tile([DH, 1], F32)
            nc.scalar.mul(bdtn[:], bdt[:], -1.0)
            # B/C replicated to pair layout, duplicated 4x along free dim
            # for the merged multiplies
            GK = 4                # pairs per merged group
            Bm2 = wp.tile([DH, GK * L], BF16)
            Cm2 = wp.tile([DH, GK * L], BF16)

            with tc.tile_pool(name="ps1", bufs=4, space="PSUM") as ps1:
                h16 = wp.tile([DM, L], F32)
                zsig = wp.tile([DH, L], F32)
                dtrT = wp.tile([DTR, L], F32)
                sg = wp.tile([DH, L], F32)
                DT = wp.tile([DH, L], BF16)  # holds -dt
                U = wp.tile([DH, L], BF16)
                xmp = [wp.tile([DH, DC - 1 + L], F32, tag=f"xmp{j}",
                               name=f"xmp{j}") for j in range(2)]
                xc16 = [wp.tile([DH, L], F32, tag=f"xc{j}", name=f"xc{j}")
                        for j in range(2)]
                bmT16 = wp.tile([DS, L], BF16)
                cmT16 = wp.tile([DS, L], BF16)
                caccf = [wp.tile([DH, L], F32, tag=f"cacc{j}",
                                 name=f"cacc{j}") for j in range(2)]
                z_psf = ps1.tile([DH, L], F32, tag="zfull", bufs=1)
                dt_psf = ps1.tile([DH, L], F32, tag="dtfull", bufs=1)
                for j in range(2):
                    nc.vector.memset(xmp[j][:, 0:DC - 1], 0.0)

                for hi, sl in enumerate(HLF):
                    HW = sl.stop - sl.start
                    # h = Wp @ x (+bp on DVE, keeping Act free for tables)
                    h_ps = ps1.tile([DM, HW], F32, tag="ps")
                    nc.tensor.matmul(h_ps[:], wpT[:, :], xt[:, sl])
                    nc.vector.tensor_scalar(
                        out=h16[:, sl], in0=h_ps[:], scalar1=bp[:],
                        scalar2=None, op0=OP.add)
                    # xm_j = W_in[chunk_j] @ h ; z gate
                    for j in range(2):
                        xm_ps = ps1.tile([DH, HW], F32, tag="ps")
                        nc.tensor.matmul(xm_ps[:],
                                         wiT[:, j * DH:(j + 1) * DH],
                                         h16[:, sl])
                        nc.scalar.copy(
                            xmp[j][:, DC - 1 + sl.start:DC - 1 + sl.stop],
                            xm_ps[:])
                    nc.tensor.matmul(z_psf[:, sl], wiT[:, 2 * DH:3 * DH],
                                     h16[:, sl])

                    # causal depthwise conv + silu -> xc16_j
                    for j in range(2):
                        c01 = wp.tile([DH, 512], F32, tag=f"c01_{j}")
                        nc.vector.tensor_scalar(
                            out=c01[:], in0=xmp[j][:, sl.start:sl.start + HW],
                            scalar1=convw[:, 4 * j:4 * j + 1], scalar2=None,
                            op0=OP.mult)
                        nc.vector.scalar_tensor_tensor(
                            out=c01[:], in0=xmp[j][:, sl.start + 1:sl.start + 1 + HW],
                            scalar=convw[:, 4 * j + 1:4 * j + 2],
                            in1=c01[:], op0=OP.mult, op1=OP.add)
                        c23 = wp.tile([DH, 512], F32, tag=f"c23_{j}")
                        nc.vector.tensor_scalar(
                            out=c23[:], in0=xmp[j][:, sl.start + 2:sl.start + 2 + HW],
                            scalar1=convw[:, 4 * j + 2:4 * j + 3], scalar2=None,
                            op0=OP.mult)
                        nc.vector.scalar_tensor_tensor(
                            out=c23[:], in0=xmp[j][:, sl.start + 3:sl.start + 3 + HW],
                            scalar=convw[:, 4 * j + 3:4 * j + 4],
                            in1=c23[:], op0=OP.mult, op1=OP.add)
                        nc.vector.tensor_tensor(out=caccf[j][:, sl],
                                                in0=c01[:], in1=c23[:],
                                                op=OP.add)

                # Silu batch: z gate + both conv outputs (one table load)
                nc.scalar.activation(zsig[:], z_psf[:], AF.Silu)
                for j in range(2):
                    nc.scalar.activation(xc16[j][:], caccf[j][:], AF.Silu,
                                         bias=convb[:, j:j + 1])

                for hi, sl in enumerate(HLF):
                    HW = sl.stop - sl.start
                    # dbc = W_x @ xc -> dtr [8,HW] and [BmT;CmT]
                    dtr_ps = ps1.tile([DTR, HW], F32, tag="ps")
                    bc_ps = ps1.tile([2 * DS, HW], F32, tag="ps")
                    for (m0, msz, out_ps) in ((0, DTR, dtr_ps),
                                              (DTR, 2 * DS, bc_ps)):
                        for j in range(2):
                            nc.tensor.matmul(
                                out_ps[:],
                                wxT[:, 136 * j + m0:136 * j + m0 + msz],
                                xc16[j][:, sl],
                                start=(j == 0), stop=(j == 1))
                    nc.vector.tensor_scalar(out=dtrT[:, sl], in0=dtr_ps[:],
                                            scalar1=0.0, scalar2=None,
                                            op0=OP.add)
                    nc.scalar.copy(bmT16[:, sl], bc_ps[0:DS, :])
                    nc.scalar.copy(cmT16[:, sl], bc_ps[DS:2 * DS, :])
                    nc.sync.dma_start(bm_scr[:, sl], bmT16[:, sl])
                    nc.sync.dma_start(cm_scr[:, sl], cmT16[:, sl])
                    # Bm2 half-chunks ride the same in-order sync queue so
                    # the first dBx never waits on slow SWDGE transfers
                    bap = bm_scr[:]
                    for k in range(GK):
                        nc.sync.dma_start(
                            Bm2[:, k * L + sl.start:k * L + sl.stop],
                            bass.AP(tensor=bap.tensor,
                                    offset=bap.offset + sl.start,
                                    ap=[[L, DS], [0, 2], [1, HW]]))
                    nc.tensor.matmul(dt_psf[:, sl], wdtT[:, :], dtrT[:, sl])

                # dt chain: -dt = ln(sigmoid(-(raw + b_dt))); one table
                # load each for Sigmoid and Ln
                nc.scalar.activation(sg[:], dt_psf[:], AF.Sigmoid,
                                     bias=bdtn[:], scale=-1.0)
                nc.scalar.activation(DT[:], sg[:], AF.Ln)
                aposp = wp.tile([DH, DS], F32)   # +exp(A_log)
                nc.scalar.activation(aposp[:], alogp[:], AF.Exp)
                for hi, sl in enumerate(HLF):
                    # U = dt * xc_own
                    nc.vector.scalar_tensor_tensor(
                        out=U[:, sl], in0=DT[:, sl], scalar=-1.0,
                        in1=xc16[0][:, sl], op0=OP.mult, op1=OP.mult)
                    nc.sync.dma_start(u_scr[:, sl], U[:, sl])
            # Cm2 replication via gpsimd SWDGE (HC consumes it later, the
            # SWDGE latency is hidden); Bm2 was filled per-half above
            sap = cm_scr[:]
            for k in range(GK):
                nc.gpsimd.dma_start(Cm2[:, k * L:(k + 1) * L], bass.AP(
                    tensor=sap.tensor, offset=sap.offset,
                    ap=[sap.ap[0], [0, 2], sap.ap[1]]))

            # ---- phase 2: selective scan, pair layout (q = 2n + j) ----
            # pair p covers channels d0=2p, d1=2p+1; partitions hold (n, j)
            # scan loop: 16 groups of 4 pairs. dtrep via PE one-hot matmul
            # into PSUM (DMA replication of dt measured as SBUF-port
            # contention). The 4 scans of a group merge into ONE scan over
            # [128, 4096]: zeroing dA's first column of pairs k>=1 makes
            # the recurrence reset exactly (h = 0*h_prev + dBx).
            LG = GK * L
            # taper both ends: small first groups let the scan start before
            # the full warmup; small last groups shorten the exit chain
            GROUPS = ([(0, 1), (1, 1), (2, 2)] +
                      [(4 + 4 * g, 4) for g in range(13)] +
                      [(56, 2), (58, 2), (60, 2), (62, 1), (63, 1)])
            with tc.tile_pool(name="psl", bufs=1, space="PSUM") as psl:
              with (
                tc.tile_pool(name="psr", bufs=3, space="PSUM") as psr,
                tc.tile_pool(name="sl", bufs=2) as slp,
              ):
                Y_ps = psl.tile([DH, L], F32, tag="Y")
                for (p0, gk) in GROUPS:
                    urep = slp.tile([DH, LG], BF16, tag="urep")
                    dAt = slp.tile([DH, LG], F32, tag="dA")
                    sap = u_scr[:]
                    for k in range(gk):
                        p = p0 + k
                        ksl = slice(k * L, (k + 1) * L)
                        dtrep = psr.tile([DH, L], F32, tag="dtrep")
                        selRp = selR[:, DH * p:DH * (p + 1)]
                        for sl in HLF:
                            nc.tensor.matmul(dtrep[:, sl], selRp, DT[:, sl])
                        nc.scalar.activation(dAt[:, ksl], dtrep[:], AF.Exp,
                                             scale=aposp[:, 0:1])
                        ueng = nc.sync if k % 2 == 0 else nc.scalar
                        ueng.dma_start(urep[:, ksl], bass.AP(
                            tensor=sap.tensor, offset=sap.offset + 2 * p * L,
                            ap=[[0, DS], [L, 2], [1, L]]))
                        if k:
                            nc.gpsimd.memset(dAt[:, k * L:k * L + 1], 0.0)
                    gsl = slice(0, gk * L)
                    # in-place: dBx overwrites urep, HC overwrites Ht
                    nc.vector.tensor_tensor(out=urep[:, gsl],
                                            in0=urep[:, gsl],
                                            in1=Bm2[:, gsl], op=OP.mult)
                    Ht = slp.tile([DH, LG], BF16, tag="H")
                    nc.vector.tensor_tensor_scan(
                        out=Ht[:, gsl], data0=dAt[:, gsl], data1=urep[:, gsl],
                        initial=0.0, op0=OP.mult, op1=OP.add)
                    HCt = Ht
                    nc.vector.tensor_tensor(out=HCt[:, gsl], in0=Ht[:, gsl],
                                            in1=Cm2[:, gsl], op=OP.mult)
                    for k in range(gk):
                        p = p0 + k
                        selp = selE[:, DH * p:DH * (p + 1)]
                        for sl in HLF:
                            nc.tensor.matmul(
                                Y_ps[:, sl], selp,
                                HCt[:, k * L + sl.start:k * L + sl.stop],
                                start=(p == 0), stop=(p == DH // 2 - 1))

          # ---- tail: gate, out-proj, pool (pipelined by t-half) ----
              y2 = wp.tile([DH, L], F32)
              y3 = wp.tile([DH, L], F32)
              trash = wp.tile([DM, L], F32)
              pooled_h = wp.tile([DM, 2], F32)
              pooled = wp.tile([DM, 1], F32)
              with tc.tile_pool(name="ps2", bufs=1, space="PSUM") as ps2:
                  out_ps = ps2.tile([DM, L], F32, tag="o")
                  for hi, sl in enumerate(HLF):
                      nc.vector.scalar_tensor_tensor(
                          out=y2[:, sl], in0=xc16[0][:, sl], scalar=dskip[:],
                          in1=Y_ps[:, sl], op0=OP.mult, op1=OP.add)
                      nc.vector.tensor_tensor(out=y3[:, sl], in0=y2[:, sl],
                                              in1=zsig[:, sl], op=OP.mult)
                      nc.tensor.matmul(out_ps[:, sl], woutT[:, :], y3[:, sl])
                      nc.scalar.activation(
                          trash[:, sl], out_ps[:, sl], AF.Identity,
                          scale=1.0 / L, accum_out=pooled_h[:, hi:hi + 1])
                  nc.vector.tensor_tensor(
                      out=pooled[:], in0=pooled_h[:, 0:1],
                      in1=pooled_h[:, 1:2], op=OP.add)
                  nc.sync.dma_start(pooled_d[:], pooled[:])

    nc.compile()
    return nc


def _core_inputs(inputs, b, half):
    f32 = np.float32
    bf16 = ml_dtypes.bfloat16
    x = np.asarray(inputs["x"], f32)
    Wp = np.asarray(inputs["Wp"], f32)
    bp = np.asarray(inputs["bp"], f32)
    W_in = np.asarray(inputs["W_in"], f32)
    conv_w = np.asarray(inputs["conv_w"], f32)
    conv_b = np.asarray(inputs["conv_b"], f32)
    W_x = np.asarray(inputs["W_x"], f32)
    W_dt = np.asarray(inputs["W_dt"], f32)
    b_dt = np.asarray(inputs["b_dt"], f32)
    A_log = np.asarray(inputs["A_log"], f32)
    Dskip = np.asarray(inputs["Dskip"], f32)
    W_out = np.asarray(inputs["W_out"], f32)

    own = slice(half * DH, half * DH + DH)
    other = slice(DH, 2 * DH) if half == 0 else slice(0, DH)
    return {
        "xt": np.ascontiguousarray(x[b]),
        "wpT": np.ascontiguousarray(Wp.T),
        "bp": np.ascontiguousarray(bp[:, None]),
        "wiT": np.concatenate(
            [W_in[0:DI][own].T, W_in[0:DI][other].T,
             W_in[DI:2 * DI][own].T], axis=1),
        "convw": np.concatenate([conv_w[own], conv_w[other]], axis=1),
        "convb": np.stack([conv_b[own], conv_b[other]], axis=1),
        "wxT": np.concatenate([W_x.T[own], W_x.T[other]], axis=1),
        "wdtT": np.ascontiguousarray(W_dt[own].T),
        "bdt": np.ascontiguousarray(b_dt[own][:, None]),
        "alogp": _alog_pairs(A_log[own]),
        "dskip": np.ascontiguousarray(Dskip[own][:, None]),
        "woutT": np.ascontiguousarray(W_out[:, own].T),
        "selE": _selE(),
        "selR": _selR(),
    }


def _alog_pairs(alog_own):
    # alogp[q, p] = A_log[own][2p + q%2, q//2]
    out = np.empty((DH, DS), np.float32)
    q = np.arange(DH)
    for p in range(DS):
        out[:, p] = alog_own[2 * p + (q % 2), q // 2]
    return out


_sel_cache = {}


def _selE():
    if "v" not in _sel_cache:
        sel = np.zeros((DH, DS * DH), np.float32)
        q = np.arange(DH)
        for p in range(DS):
            sel[q, DH * p + 2 * p + (q % 2)] = 1.0
        _sel_cache["v"] = sel.astype(ml_dtypes.bfloat16)
    return _sel_cache["v"]


def _selR():
    if "r" not in _sel_cache:
        sel = np.zeros((DH, DS * DH), np.float32)
        q = np.arange(DH)
        for p in range(DS):
            sel[2 * p + (q % 2), DH * p + q] = 1.0
        _sel_cache["r"] = sel.astype(ml_dtypes.bfloat16)
    return _sel_cache["r"]


def kernel(**inputs) -> np.ndarray:
    if "nc" not in _cache:
        _cache["nc"] = _build()
    nc = _cache["nc"]

    in_maps = [_core_inputs(inputs, c // 2, c % 2) for c in range(8)]
    res = run_bass_kernel_spmd(nc, in_maps, core_ids=list(range(8)))

    pooled = np.zeros((B, DM), np.float32)
    for c in range(8):
        pooled[c // 2] += res.results[c]["pooled"][:, 0]

    # classifier head (host: BatchNorm couples all batches; ~300 flops)
    f32 = np.float32
    W1 = np.asarray(inputs["W1"], f32)
    b1 = np.asarray(inputs["b1"], f32)
    gamma = np.asarray(inputs["gamma"], f32)
    beta = np.asarray(inputs["beta"], f32)
    W2 = np.asarray(inputs["W2"], f32)
    b2 = np.asarray(inputs["b2"], f32)
    h1 = pooled @ W1.T + b1
    mu = h1.mean(axis=0)
    var = h1.var(axis=0)
    h1 = (h1 - mu) / np.sqrt(var + EPS) * gamma + beta
    h1 = np.maximum(h1, 0.0)
    return (h1 @ W2.T + b2).astype(np.float32)



# revision 40
# speedup vs baseline: 1.0012x; 1.0012x over previous
"""Trainium2 Bass kernel for nn_BASE_MAMBA_14018773254552.

Mamba block (d_model=128, d_inner=256, d_state=64, d_conv=4, L=1024, B=4)
+ input proj + classifier head.

Sharding: 8 cores = 4 batches x 2 d_inner-halves (128 channels each).
Each core computes its batch's full front-end (input proj, in_proj, conv,
x_proj) feature-major ([feature, time] tiles), then the selective scan for
its 128-channel half, and the partial out-proj + mean-pool. The host sums
the two channel-half partials per batch and runs the tiny classifier
(BatchNorm couples batches, so it cannot live on one core).

Scan-loop design (DVE-bound; see session notes):
- pair p = channels (2p, 2p+1); scan tile partitions hold (state n, j).
- pairs are merged into groups of up to 4: ONE tensor_tensor_scan over
  [128, 4096] replaces 4 scans -- exact because dA's first column of
  each inner pair is zeroed (h = 0*h_prev + dBx resets the recurrence).
  Group sizes taper (1,1,2,4...4,2,2,2,1,1) so the first scan starts
  before full warmup and the last group's serial exit chain is short.
- dBx/HC multiplies run merged and in-place (urep/Ht buffers).
- dt replication runs on PE via one-hot selR matmuls into PSUM (a DMA
  replication of dt measured slower: SBUF write-port contention slows
  every DVE op by ~20%).
- u replication via DMA from a DRAM bounce, alternating the two HWDGE
  queues (sync + scalar-triggered); B/C replication via gpsimd SWDGE.
- ACT tables reload on every function switch (1.28us), so table-using
  activations are batched by function: Silu x3, Sigmoid, Ln, then Exp
  for all dA tiles.

Self-contained: hardcodes all shapes; builds + compiles the Bass program
once per process and runs it on cores 0-7 via run_bass_kernel_spmd.
"""
import numpy as np

try:
    import concourse.bacc as bacc
except ImportError:  # pragma: no cover - path fallback
    import sys
    for _p in ("/opt/trn_rl_repo", "/root/.axon_site/_ro/trn_rl_repo"):
        if _p not in sys.path:
            sys.path.insert(0, _p)
    import concourse.bacc as bacc

import ml_dtypes
import concourse.bass as bass
import concourse.mybir as mybir
import concourse.tile as tile
from concourse.bass_utils import run_bass_kernel_spmd

F32 = mybir.dt.float32
BF16 = mybir.dt.bfloat16
AF = mybir.ActivationFunctionType
OP = mybir.AluOpType

B, L, CIN = 4, 1024, 20
DM, DS, DC = 128, 64, 4
DI = 256
DTR = 8
DH = 128          # channels per core (d_inner half)
EPS = 1e-5

_cache = {}


def _build():
    nc = bacc.Bacc("TRN2", target_bir_lowering=False, debug=False, num_devices=8)

    # ---- I/O ----
    xt_d = nc.dram_tensor("xt", [CIN, L], F32, kind="ExternalInput")
    wpT_d = nc.dram_tensor("wpT", [CIN, DM], F32, kind="ExternalInput")
    bp_d = nc.dram_tensor("bp", [DM, 1], F32, kind="ExternalInput")
    wiT_d = nc.dram_tensor("wiT", [DM, 3 * DH], F32, kind="ExternalInput")
    convw_d = nc.dram_tensor("convw", [DH, 2 * DC], F32, kind="ExternalInput")
    convb_d = nc.dram_tensor("convb", [DH, 2], F32, kind="ExternalInput")
    wxT_d = nc.dram_tensor("wxT", [DH, 2 * 136], F32, kind="ExternalInput")
    wdtT_d = nc.dram_tensor("wdtT", [DTR, DH], F32, kind="ExternalInput")
    bdt_d = nc.dram_tensor("bdt", [DH, 1], F32, kind="ExternalInput")
    alogp_d = nc.dram_tensor("alogp", [DH, DS], F32, kind="ExternalInput")
    dskip_d = nc.dram_tensor("dskip", [DH, 1], F32, kind="ExternalInput")
    woutT_d = nc.dram_tensor("woutT", [DH, DM], BF16, kind="ExternalInput")
    selE_d = nc.dram_tensor("selE", [DH, DS * DH], BF16, kind="ExternalInput")
    selR_d = nc.dram_tensor("selR", [DH, DS * DH], BF16, kind="ExternalInput")
    pooled_d = nc.dram_tensor("pooled", [DM, 1], F32, kind="ExternalOutput")
    u_scr = nc.dram_tensor("u_scr", [DH, L], BF16)
    dt_scr = nc.dram_tensor("dt_scr", [DH, L], BF16)
    bm_scr = nc.dram_tensor("bm_scr", [DS, L], BF16)
    cm_scr = nc.dram_tensor("cm_scr", [DS, L], BF16)

    with tile.TileContext(nc) as tc:
        with (
            tc.tile_pool(name="const", bufs=1) as cp,
            tc.tile_pool(name="work", bufs=1) as wp,
        ):
            # ---- load params ----
            xt = cp.tile([CIN, L], F32)
            wpT = cp.tile([CIN, DM], F32)
            bp = cp.tile([DM, 1], F32)
            wiT = cp.tile([DM, 3 * DH], F32)
            convw = cp.tile([DH, 2 * DC], F32)
            convb = cp.tile([DH, 2], F32)
            wxT = cp.tile([DH, 2 * 136], F32)
            wdtT = cp.tile([DTR, DH], F32)
            bdt = cp.tile([DH, 1], F32)
            alogp = cp.tile([DH, DS], F32)
            dskip = cp.tile([DH, 1], F32)
            woutT = cp.tile([DH, DM], BF16)
            selE = cp.tile([DH, DS * DH], BF16)
            selR = cp.tile([DH, DS * DH], BF16)
            for t_, d_ in [(xt, xt_d), (wpT, wpT_d), (bp, bp_d), (wiT, wiT_d),
                           (convw, convw_d), (convb, convb_d), (wxT, wxT_d),
                           (wdtT, wdtT_d), (bdt, bdt_d), (alogp, alogp_d),
                           (dskip, dskip_d), (woutT, woutT_d), (selE, selE_d),
                           (selR, selR_d)]:
                nc.sync.dma_start(t_[:], d_[:])

            HLF = (slice(0, 512), slice(512, 1024))

            # ---- phase 1: front-end ----
            # ACT tables reload on every function switch (1.28us) so all
            # table-using ACTs run full-length, batched by function; the
            # DVE/PE/copy stages pipeline in time-halves.
            bdtn = wp.tile([DH, 1], F32)
            nc.scalar.mul(bdtn[:], bdt[:], -1.0)
            # B/C replicated to pair layout, duplicated 4x along free dim
            # for the merged multiplies
            GK = 4                # pairs per merged group
            Bm2 = wp.tile([DH, GK * L], BF16)
            Cm2 = wp.tile([DH, GK * L], BF16)

            with tc.tile_pool(name="ps1", bufs=4, space="PSUM") as ps1:
                h16 = wp.tile([DM, L], F32)
                zsig = wp.tile([DH, L], F32)
                dtrT = wp.tile([DTR, L], F32)
                sg = wp.tile([DH, L], F32)
                DT = wp.tile([DH, L], BF16)  # holds -dt
                U = wp.tile([DH, L], BF16)
                xmp = [wp.tile([DH, DC - 1 + L], F32, tag=f"xmp{j}",
                               name=f"xmp{j}") for j in range(2)]
                xc16 = [wp.tile([DH, L], F32, tag=f"xc{j}", name=f"xc{j}")
                        for j in range(2)]
                bmT16 = wp.tile([DS, L], BF16)
                cmT16 = wp.tile([DS, L], BF16)
                caccf = [wp.tile([DH, L], F32, tag=f"cacc{j}",
                                 name=f"cacc{j}") for j in range(2)]
                z_psf = ps1.tile([DH, L], F32, tag="zfull", bufs=1)
                dt_psf = ps1.tile([DH, L], F32, tag="dtfull", bufs=1)
                for j in range(2):
                    nc.vector.memset(xmp[j][:, 0:DC - 1], 0.0)

                for hi, sl in enumerate(HLF):
                    HW = sl.stop - sl.start
                    # h = Wp @ x (+bp on DVE, keeping Act free for tables)
                    h_ps = ps1.tile([DM, HW], F32, tag="ps")
                    nc.tensor.matmul(h_ps[:], wpT[:, :], xt[:, sl])
                    nc.vector.tensor_scalar(
                        out=h16[:, sl], in0=h_ps[:], scalar1=bp[:],
                        scalar2=None, op0=OP.add)
                    # xm_j = W_in[chunk_j] @ h ; z gate
                    for j in range(2):
                        xm_ps = ps1.tile([DH, HW], F32, tag="ps")
                        nc.tensor.matmul(xm_ps[:],
                                         wiT[:, j * DH:(j + 1) * DH],
                                         h16[:, sl])
                        nc.scalar.copy(
                            xmp[j][:, DC - 1 + sl.start:DC - 1 + sl.stop],
                            xm_ps[:])
                    nc.tensor.matmul(z_psf[:, sl], wiT[:, 2 * DH:3 * DH],
                                     h16[:, sl])

                    # causal depthwise conv + silu -> xc16_j
                    for j in range(2):
                        c01 = wp.tile([DH, 512], F32, tag=f"c01_{j}")
                        nc.vector.tensor_scalar(
                            out=c01[:], in0=xmp[j][:, sl.start:sl.start + HW],
                            scalar1=convw[:, 4 * j:4 * j + 1], scalar2=None,
                            op0=OP.mult)
                        nc.vector.scalar_tensor_tensor(
                            out=c01[:], in0=xmp[j][:, sl.start + 1:sl.start + 1 + HW],
                            scalar=convw[:, 4 * j + 1:4 * j + 2],
                            in1=c01[:], op0=OP.mult, op1=OP.add)
                        c23 = wp.tile([DH, 512], F32, tag=f"c23_{j}")
                        nc.vector.tensor_scalar(
                            out=c23[:], in0=xmp[j][:, sl.start + 2:sl.start + 2 + HW],
                            scalar1=convw[:, 4 * j + 2:4 * j + 3], scalar2=None,
                            op0=OP.mult)
                        nc.vector.scalar_tensor_tensor(
                            out=c23[:], in0=xmp[j][:, sl.start + 3:sl.start + 3 + HW],
                            scalar=convw[:, 4 * j + 3:4 * j + 4],
                            in1=c23[:], op0=OP.mult, op1=OP.add)
                        nc.vector.tensor_tensor(out=caccf[j][:, sl],
                                                in0=c01[:], in1=c23[:],
                                                op=OP.add)

                # Silu batch: z gate + both conv outputs (one table load)
                nc.scalar.activation(zsig[:], z_psf[:], AF.Silu)
                for j in range(2):
                    nc.scalar.activation(xc16[j][:], caccf[j][:], AF.Silu,
                                         bias=convb[:, j:j + 1])

                for hi, sl in enumerate(HLF):
                    HW = sl.stop - sl.start
                    # dbc = W_x @ xc -> dtr [8,HW] and [BmT;CmT]
                    dtr_ps = ps1.tile([DTR, HW], F32, tag="ps")
                    bc_ps = ps1.tile([2 * DS, HW], F32, tag="ps")
                    for (m0, msz, out_ps) in ((0, DTR, dtr_ps),
                                              (DTR, 2 * DS, bc_ps)):
                        for j in range(2):
                            nc.tensor.matmul(
                                out_ps[:],
                                wxT[:, 136 * j + m0:136 * j + m0 + msz],
                                xc16[j][:, sl],
                                start=(j == 0), stop=(j == 1))
                    nc.vector.tensor_scalar(out=dtrT[:, sl], in0=dtr_ps[:],
                                            scalar1=0.0, scalar2=None,
                                            op0=OP.add)
                    nc.scalar.copy(bmT16[:, sl], bc_ps[0:DS, :])
                    nc.scalar.copy(cmT16[:, sl], bc_ps[DS:2 * DS, :])
                    nc.sync.dma_start(bm_scr[:, sl], bmT16[:, sl])
                    nc.sync.dma_start(cm_scr[:, sl], cmT16[:, sl])
                    # Bm2 half-chunks ride the same in-order sync queue so
                    # the first dBx never waits on slow SWDGE transfers
                    bap = bm_scr[:]
                    for k in range(GK):
                        nc.sync.dma_start(
                            Bm2[:, k * L + sl.start:k * L + sl.stop],
                            bass.AP(tensor=bap.tensor,
                                    offset=bap.offset + sl.start,
                                    ap=[[L, DS], [0, 2], [1, HW]]))
                    nc.tensor.matmul(dt_psf[:, sl], wdtT[:, :], dtrT[:, sl])

                # dt chain: -dt = ln(sigmoid(-(raw + b_dt))); one table
                # load each for Sigmoid and Ln
                nc.scalar.activation(sg[:], dt_psf[:], AF.Sigmoid,
                                     bias=bdtn[:], scale=-1.0)
                nc.scalar.activation(DT[:], sg[:], AF.Ln)
                aposp = wp.tile([DH, DS], F32)   # +exp(A_log)
                nc.scalar.activation(aposp[:], alogp[:], AF.Exp)
                for hi, sl in enumerate(HLF):
                    # U = dt * xc_own
                    nc.vector.scalar_tensor_tensor(
                        out=U[:, sl], in0=DT[:, sl], scalar=-1.0,
                        in1=xc16[0][:, sl], op0=OP.mult, op1=OP.mult)
                    nc.sync.dma_start(u_scr[:, sl], U[:, sl])
            # Cm2 replication via gpsimd SWDGE (HC consumes it later, the
            # SWDGE latency is hidden); Bm2 was filled per-half above
            sap = cm_scr[:]
            for k in range(GK):
                nc.gpsimd.dma_start(Cm2[:, k * L:(k + 1) * L], bass.AP(
                    tensor=sap.tensor, offset=sap.offset,
                    ap=[sap.ap[0], [0, 2], sap.ap[1]]))

            # ---- phase 2: selective scan, pair layout (q = 2n + j) ----
            # pair p covers channels d0=2p, d1=2p+1; partitions hold (n, j)
            # scan loop: 16 groups of 4 pairs. dtrep via PE one-hot matmul
            # into PSUM (DMA replication of dt measured as SBUF-port
            # contention). The 4 scans of a group merge into ONE scan over
            # [128, 4096]: zeroing dA's first column of pairs k>=1 makes
            # the recurrence reset exactly (h = 0*h_prev + dBx).
            LG = GK * L
            # taper both ends: small first groups let the scan start before
            # the full warmup; small last groups shorten the exit chain
            GROUPS = ([(0, 1), (1, 1), (2, 2)] +
                      [(4 + 4 * g, 4) for g in range(13)] +
                      [(56, 2), (58, 2), (60, 2), (62, 1), (63, 1)])
            with tc.tile_pool(name="psl", bufs=1, space="PSUM") as psl:
              with (
                tc.tile_pool(name="psr", bufs=3, space="PSUM") as psr,
                tc.tile_pool(name="sl", bufs=2) as slp,
              ):
                Y_ps = psl.tile([DH, L], F32, tag="Y")
                for (p0, gk) in GROUPS:
                    urep = slp.tile([DH, LG], BF16, tag="urep")
                    dAt = slp.tile([DH, LG], F32, tag="dA")
                    sap = u_scr[:]
                    for k in range(gk):
                        p = p0 + k
                        ksl = slice(k * L, (k + 1) * L)
                        dtrep = psr.tile([DH, L], F32, tag="dtrep")
                        selRp = selR[:, DH * p:DH * (p + 1)]
                        for sl in HLF:
                            nc.tensor.matmul(dtrep[:, sl], selRp, DT[:, sl])
                        nc.scalar.activation(dAt[:, ksl], dtrep[:], AF.Exp,
                                             scale=aposp[:, 0:1])
                        ueng = nc.sync if k % 2 == 0 else nc.scalar
                        ueng.dma_start(urep[:, ksl], bass.AP(
                            tensor=sap.tensor, offset=sap.offset + 2 * p * L,
                            ap=[[0, DS], [L, 2], [1, L]]))
                        if k:
                            nc.gpsimd.memset(dAt[:, k * L:k * L + 1], 0.0)
                    gsl = slice(0, gk * L)
                    # in-place: dBx overwrites urep, HC overwrites Ht
                    nc.vector.tensor_tensor(out=urep[:, gsl],
                                            in0=urep[:, gsl],
                                            in1=Bm2[:, gsl], op=OP.mult)
                    Ht = slp.tile([DH, LG], BF16, tag="H")
                    nc.vector.tensor_tensor_scan(
                        out=Ht[:, gsl], data0=dAt[:, gsl], data1=urep[:, gsl],
                        initial=0.0, op0=OP.mult, op1=OP.add)
                    HCt = Ht
                    nc.vector.tensor_tensor(out=HCt[:, gsl], in0=Ht[:, gsl],
                                            in1=Cm2[:, gsl], op=OP.mult)
                    for k in range(gk):
                        p = p0 + k
                        selp = selE[:, DH * p:DH * (p + 1)]
                        for sl in HLF:
                            nc.tensor.matmul(
                                Y_ps[:, sl], selp,
                                HCt[:, k * L + sl.start:k * L + sl.stop],
                                start=(p == 0), stop=(p == DH // 2 - 1))

          # ---- tail: gate, out-proj, pool (pipelined by t-half) ----
              y2 = wp.tile([DH, L], F32)
              y3 = wp.tile([DH, L], BF16)
              trash = wp.tile([DM, L], F32)
              pooled_h = wp.tile([DM, 2], F32)
              pooled = wp.tile([DM, 1], F32)
              with tc.tile_pool(name="ps2", bufs=1, space="PSUM") as ps2:
                  out_ps = ps2.tile([DM, L], F32, tag="o")
                  for hi, sl in enumerate(HLF):
                      nc.vector.scalar_tensor_tensor(
                          out=y2[:, sl], in0=xc16[0][:, sl], scalar=dskip[:],
                          in1=Y_ps[:, sl], op0=OP.mult, op1=OP.add)
                      nc.vector.tensor_tensor(out=y3[:, sl], in0=y2[:, sl],
                                              in1=zsig[:, sl], op=OP.mult)
                      nc.tensor.matmul(out_ps[:, sl], woutT[:, :], y3[:, sl])
                      nc.scalar.activation(
                          trash[:, sl], out_ps[:, sl], AF.Identity,
                          scale=1.0 / L, accum_out=pooled_h[:, hi:hi + 1])
                  nc.vector.tensor_tensor(
                      out=pooled[:], in0=pooled_h[:, 0:1],
                      in1=pooled_h[:, 1:2], op=OP.add)
                  nc.sync.dma_start(pooled_d[:], pooled[:])

    nc.compile()
    return nc


def _core_inputs(inputs, b, half):
    f32 = np.float32
    bf16 = ml_dtypes.bfloat16
    x = np.asarray(inputs["x"], f32)
    Wp = np.asarray(inputs["Wp"], f32)
    bp = np.asarray(inputs["bp"], f32)
    W_in = np.asarray(inputs["W_in"], f32)
    conv_w = np.asarray(inputs["conv_w"], f32)
    conv_b = np.asarray(inputs["conv_b"], f32)
    W_x = np.asarray(inputs["W_x"], f32)
    W_dt = np.asarray(inputs["W_dt"], f32)
    b_dt = np.asarray(inputs["b_dt"], f32)
    A_log = np.asarray(inputs["A_log"], f32)
    Dskip = np.asarray(inputs["Dskip"], f32)
    W_out = np.asarray(inputs["W_out"], f32)

    own = slice(half * DH, half * DH + DH)
    other = slice(DH, 2 * DH) if half == 0 else slice(0, DH)
    return {
        "xt": np.ascontiguousarray(x[b]),
        "wpT": np.ascontiguousarray(Wp.T),
        "bp": np.ascontiguousarray(bp[:, None]),
        "wiT": np.concatenate(
            [W_in[0:DI][own].T, W_in[0:DI][other].T,
             W_in[DI:2 * DI][own].T], axis=1),
        "convw": np.concatenate([conv_w[own], conv_w[other]], axis=1),
        "convb": np.stack([conv_b[own], conv_b[other]], axis=1),
        "wxT": np.concatenate([W_x.T[own], W_x.T[other]], axis=1),
        "wdtT": np.ascontiguousarray(W_dt[own].T),
        "bdt": np.ascontiguousarray(b_dt[own][:, None]),
        "alogp": _alog_pairs(A_log[own]),
        "dskip": np.ascontiguousarray(Dskip[own][:, None]),
        "woutT": np.ascontiguousarray(W_out[:, own].T).astype(bf16),
        "selE": _selE(),
        "selR": _selR(),
    }


def _alog_pairs(alog_own):
    # alogp[q, p] = A_log[own][2p + q%2, q//2]
    out = np.empty((DH, DS), np.float32)
    q = np.arange(DH)
    for p in range(DS):
        out[:, p] = alog_own[2 * p + (q % 2), q // 2]
    return out


_sel_cache = {}


def _selE():
    if "v" not in _sel_cache:
        sel = np.zeros((DH, DS * DH), np.float32)
        q = np.arange(DH)
        for p in range(DS):
            sel[q, DH * p + 2 * p + (q % 2)] = 1.0
        _sel_cache["v"] = sel.astype(ml_dtypes.bfloat16)
    return _sel_cache["v"]


def _selR():
    if "r" not in _sel_cache:
        sel = np.zeros((DH, DS * DH), np.float32)
        q = np.arange(DH)
        for p in range(DS):
            sel[2 * p + (q % 2), DH * p + q] = 1.0
        _sel_cache["r"] = sel.astype(ml_dtypes.bfloat16)
    return _sel_cache["r"]


def kernel(**inputs) -> np.ndarray:
    if "nc" not in _cache:
        _cache["nc"] = _build()
    nc = _cache["nc"]

    in_maps = [_core_inputs(inputs, c // 2, c % 2) for c in range(8)]
    res = run_bass_kernel_spmd(nc, in_maps, core_ids=list(range(8)))

    pooled = np.zeros((B, DM), np.float32)
    for c in range(8):
        pooled[c // 2] += res.results[c]["pooled"][:, 0]

    # classifier head (host: BatchNorm couples all batches; ~300 flops)
    f32 = np.float32
    W1 = np.asarray(inputs["W1"], f32)
    b1 = np.asarray(inputs["b1"], f32)
    gamma = np.asarray(inputs["gamma"], f32)
    beta = np.asarray(inputs["beta"], f32)
    W2 = np.asarray(inputs["W2"], f32)
    b2 = np.asarray(inputs["b2"], f32)
    h1 = pooled @ W1.T + b1
    mu = h1.mean(axis=0)
    var = h1.var(axis=0)
    h1 = (h1 - mu) / np.sqrt(var + EPS) * gamma + beta
    h1 = np.maximum(h1, 0.0)
    return (h1 @ W2.T + b2).astype(np.float32)



# revision 41
# speedup vs baseline: 1.0074x; 1.0062x over previous
"""Trainium2 Bass kernel for nn_BASE_MAMBA_14018773254552.

Mamba block (d_model=128, d_inner=256, d_state=64, d_conv=4, L=1024, B=4)
+ input proj + classifier head.

Sharding: 8 cores = 4 batches x 2 d_inner-halves (128 channels each).
Each core computes its batch's full front-end (input proj, in_proj, conv,
x_proj) feature-major ([feature, time] tiles), then the selective scan for
its 128-channel half, and the partial out-proj + mean-pool. The host sums
the two channel-half partials per batch and runs the tiny classifier
(BatchNorm couples batches, so it cannot live on one core).

Scan-loop design (DVE-bound; see session notes):
- pair p = channels (2p, 2p+1); scan tile partitions hold (state n, j).
- pairs are merged into groups of up to 4: ONE tensor_tensor_scan over
  [128, 4096] replaces 4 scans -- exact because dA's first column of
  each inner pair is zeroed (h = 0*h_prev + dBx resets the recurrence).
  Group sizes taper (1,1,2,4...4,2,2,2,1,1) so the first scan starts
  before full warmup and the last group's serial exit chain is short.
- dBx/HC multiplies run merged and in-place (urep/Ht buffers).
- dt replication runs on PE via one-hot selR matmuls into PSUM (a DMA
  replication of dt measured slower: SBUF write-port contention slows
  every DVE op by ~20%).
- u replication via DMA from a DRAM bounce, alternating the two HWDGE
  queues (sync + scalar-triggered); B/C replication via gpsimd SWDGE.
- ACT tables reload on every function switch (1.28us), so table-using
  activations are batched by function: Silu x3, Sigmoid, Ln, then Exp
  for all dA tiles.

Self-contained: hardcodes all shapes; builds + compiles the Bass program
once per process and runs it on cores 0-7 via run_bass_kernel_spmd.
"""
import numpy as np

try:
    import concourse.bacc as bacc
except ImportError:  # pragma: no cover - path fallback
    import sys
    for _p in ("/opt/trn_rl_repo", "/root/.axon_site/_ro/trn_rl_repo"):
        if _p not in sys.path:
            sys.path.insert(0, _p)
    import concourse.bacc as bacc

import ml_dtypes
import concourse.bass as bass
import concourse.mybir as mybir
import concourse.tile as tile
from concourse.bass_utils import run_bass_kernel_spmd

F32 = mybir.dt.float32
BF16 = mybir.dt.bfloat16
AF = mybir.ActivationFunctionType
OP = mybir.AluOpType

B, L, CIN = 4, 1024, 20
DM, DS, DC = 128, 64, 4
DI = 256
DTR = 8
DH = 128          # channels per core (d_inner half)
EPS = 1e-5

_cache = {}


def _build():
    nc = bacc.Bacc("TRN2", target_bir_lowering=False, debug=False, num_devices=8)

    # ---- I/O ----
    xt_d = nc.dram_tensor("xt", [CIN, L], F32, kind="ExternalInput")
    wpT_d = nc.dram_tensor("wpT", [CIN, DM], F32, kind="ExternalInput")
    bp_d = nc.dram_tensor("bp", [DM, 1], F32, kind="ExternalInput")
    wiT_d = nc.dram_tensor("wiT", [DM, 3 * DH], F32, kind="ExternalInput")
    convw_d = nc.dram_tensor("convw", [DH, 2 * DC], F32, kind="ExternalInput")
    convb_d = nc.dram_tensor("convb", [DH, 2], F32, kind="ExternalInput")
    wxT_d = nc.dram_tensor("wxT", [DH, 2 * 136], F32, kind="ExternalInput")
    wdtT_d = nc.dram_tensor("wdtT", [DTR, DH], F32, kind="ExternalInput")
    bdt_d = nc.dram_tensor("bdt", [DH, 1], F32, kind="ExternalInput")
    alogp_d = nc.dram_tensor("alogp", [DH, DS], F32, kind="ExternalInput")
    dskip_d = nc.dram_tensor("dskip", [DH, 1], F32, kind="ExternalInput")
    woutT_d = nc.dram_tensor("woutT", [DH, DM], F32, kind="ExternalInput")
    selE_d = nc.dram_tensor("selE", [DH, DS * DH], BF16, kind="ExternalInput")
    selR_d = nc.dram_tensor("selR", [DH, DS * DH], BF16, kind="ExternalInput")
    pooled_d = nc.dram_tensor("pooled", [DM, 1], F32, kind="ExternalOutput")
    u_scr = nc.dram_tensor("u_scr", [DH, L], BF16)
    dt_scr = nc.dram_tensor("dt_scr", [DH, L], BF16)
    bm_scr = nc.dram_tensor("bm_scr", [DS, L], BF16)
    cm_scr = nc.dram_tensor("cm_scr", [DS, L], BF16)

    with tile.TileContext(nc) as tc:
        with (
            tc.tile_pool(name="const", bufs=1) as cp,
            tc.tile_pool(name="work", bufs=1) as wp,
        ):
            # ---- load params ----
            xt = cp.tile([CIN, L], F32)
            wpT = cp.tile([CIN, DM], F32)
            bp = cp.tile([DM, 1], F32)
            wiT = cp.tile([DM, 3 * DH], F32)
            convw = cp.tile([DH, 2 * DC], F32)
            convb = cp.tile([DH, 2], F32)
            wxT = cp.tile([DH, 2 * 136], F32)
            wdtT = cp.tile([DTR, DH], F32)
            bdt = cp.tile([DH, 1], F32)
            alogp = cp.tile([DH, DS], F32)
            dskip = cp.tile([DH, 1], F32)
            woutT = cp.tile([DH, DM], F32)
            selE = cp.tile([DH, DS * DH], BF16)
            selR = cp.tile([DH, DS * DH], BF16)
            for t_, d_ in [(xt, xt_d), (wpT, wpT_d), (bp, bp_d), (wiT, wiT_d),
                           (convw, convw_d), (convb, convb_d), (wxT, wxT_d),
                           (wdtT, wdtT_d), (bdt, bdt_d), (alogp, alogp_d),
                           (dskip, dskip_d), (woutT, woutT_d), (selE, selE_d),
                           (selR, selR_d)]:
                nc.sync.dma_start(t_[:], d_[:])

            HLF = (slice(0, 512), slice(512, 1024))

            # ---- phase 1: front-end ----
            # ACT tables reload on every function switch (1.28us) so all
            # table-using ACTs run full-length, batched by function; the
            # DVE/PE/copy stages pipeline in time-halves.
            bdtn = wp.tile([DH, 1], F32)
            nc.scalar.mul(bdtn[:], bdt[:], -1.0)
            # B/C replicated to pair layout, duplicated 4x along free dim
            # for the merged multiplies
            GK = 4                # pairs per merged group
            Bm2 = wp.tile([DH, GK * L], BF16)
            Cm2 = wp.tile([DH, GK * L], BF16)

            with tc.tile_pool(name="ps1", bufs=4, space="PSUM") as ps1:
                h16 = wp.tile([DM, L], F32)
                zsig = wp.tile([DH, L], F32)
                dtrT = wp.tile([DTR, L], F32)
                sg = wp.tile([DH, L], F32)
                DT = wp.tile([DH, L], BF16)  # holds -dt
                U = wp.tile([DH, L], BF16)
                xmp = [wp.tile([DH, DC - 1 + L], F32, tag=f"xmp{j}",
                               name=f"xmp{j}") for j in range(2)]
                xc16 = [wp.tile([DH, L], F32, tag=f"xc{j}", name=f"xc{j}")
                        for j in range(2)]
                bmT16 = wp.tile([DS, L], BF16)
                cmT16 = wp.tile([DS, L], BF16)
                caccf = [wp.tile([DH, L], F32, tag=f"cacc{j}",
                                 name=f"cacc{j}") for j in range(2)]
                z_psf = ps1.tile([DH, L], F32, tag="zfull", bufs=1)
                dt_psf = ps1.tile([DH, L], F32, tag="dtfull", bufs=1)
                for j in range(2):
                    nc.vector.memset(xmp[j][:, 0:DC - 1], 0.0)

                for hi, sl in enumerate(HLF):
                    HW = sl.stop - sl.start
                    # h = Wp @ x (+bp on DVE, keeping Act free for tables)
                    h_ps = ps1.tile([DM, HW], F32, tag="ps")
                    nc.tensor.matmul(h_ps[:], wpT[:, :], xt[:, sl])
                    nc.vector.tensor_scalar(
                        out=h16[:, sl], in0=h_ps[:], scalar1=bp[:],
                        scalar2=None, op0=OP.add)
                    # xm_j = W_in[chunk_j] @ h ; z gate
                    for j in range(2):
                        xm_ps = ps1.tile([DH, HW], F32, tag="ps")
                        nc.tensor.matmul(xm_ps[:],
                                         wiT[:, j * DH:(j + 1) * DH],
                                         h16[:, sl])
                        nc.scalar.copy(
                            xmp[j][:, DC - 1 + sl.start:DC - 1 + sl.stop],
                            xm_ps[:])
                    nc.tensor.matmul(z_psf[:, sl], wiT[:, 2 * DH:3 * DH],
                                     h16[:, sl])

                    # causal depthwise conv + silu -> xc16_j
                    for j in range(2):
                        c01 = wp.tile([DH, 512], F32, tag=f"c01_{j}")
                        nc.vector.tensor_scalar(
                            out=c01[:], in0=xmp[j][:, sl.start:sl.start + HW],
                            scalar1=convw[:, 4 * j:4 * j + 1], scalar2=None,
                            op0=OP.mult)
                        nc.vector.scalar_tensor_tensor(
                            out=c01[:], in0=xmp[j][:, sl.start + 1:sl.start + 1 + HW],
                            scalar=convw[:, 4 * j + 1:4 * j + 2],
                            in1=c01[:], op0=OP.mult, op1=OP.add)
                        c23 = wp.tile([DH, 512], F32, tag=f"c23_{j}")
                        nc.vector.tensor_scalar(
                            out=c23[:], in0=xmp[j][:, sl.start + 2:sl.start + 2 + HW],
                            scalar1=convw[:, 4 * j + 2:4 * j + 3], scalar2=None,
                            op0=OP.mult)
                        nc.vector.scalar_tensor_tensor(
                            out=c23[:], in0=xmp[j][:, sl.start + 3:sl.start + 3 + HW],
                            scalar=convw[:, 4 * j + 3:4 * j + 4],
                            in1=c23[:], op0=OP.mult, op1=OP.add)
                        nc.vector.tensor_tensor(out=caccf[j][:, sl],
                                                in0=c01[:], in1=c23[:],
                                                op=OP.add)

                # Silu batch: z gate + both conv outputs (one table load)
                nc.scalar.activation(zsig[:], z_psf[:], AF.Silu)
                for j in range(2):
                    nc.scalar.activation(xc16[j][:], caccf[j][:], AF.Silu,
                                         bias=convb[:, j:j + 1])

                for hi, sl in enumerate(HLF):
                    HW = sl.stop - sl.start
                    # dbc = W_x @ xc -> dtr [8,HW] and [BmT;CmT]
                    dtr_ps = ps1.tile([DTR, HW], F32, tag="ps")
                    bc_ps = ps1.tile([2 * DS, HW], F32, tag="ps")
                    for (m0, msz, out_ps) in ((0, DTR, dtr_ps),
                                              (DTR, 2 * DS, bc_ps)):
                        for j in range(2):
                            nc.tensor.matmul(
                                out_ps[:],
                                wxT[:, 136 * j + m0:136 * j + m0 + msz],
                                xc16[j][:, sl],
                                start=(j == 0), stop=(j == 1))
                    nc.vector.tensor_scalar(out=dtrT[:, sl], in0=dtr_ps[:],
                                            scalar1=0.0, scalar2=None,
                                            op0=OP.add)
                    nc.scalar.copy(bmT16[:, sl], bc_ps[0:DS, :])
                    nc.scalar.copy(cmT16[:, sl], bc_ps[DS:2 * DS, :])
                    nc.sync.dma_start(bm_scr[:, sl], bmT16[:, sl])
                    nc.sync.dma_start(cm_scr[:, sl], cmT16[:, sl])
                    # Bm2 half-chunks ride the same in-order sync queue so
                    # the first dBx never waits on slow SWDGE transfers
                    bap = bm_scr[:]
                    for k in range(GK):
                        nc.sync.dma_start(
                            Bm2[:, k * L + sl.start:k * L + sl.stop],
                            bass.AP(tensor=bap.tensor,
                                    offset=bap.offset + sl.start,
                                    ap=[[L, DS], [0, 2], [1, HW]]))
                    nc.tensor.matmul(dt_psf[:, sl], wdtT[:, :], dtrT[:, sl])

                # dt chain: -dt = ln(sigmoid(-(raw + b_dt))); one table
                # load each for Sigmoid and Ln
                nc.scalar.activation(sg[:], dt_psf[:], AF.Sigmoid,
                                     bias=bdtn[:], scale=-1.0)
                nc.scalar.activation(DT[:], sg[:], AF.Ln)
                aposp = wp.tile([DH, DS], F32)   # +exp(A_log)
                nc.scalar.activation(aposp[:], alogp[:], AF.Exp)
                for hi, sl in enumerate(HLF):
                    # U = dt * xc_own
                    nc.vector.scalar_tensor_tensor(
                        out=U[:, sl], in0=DT[:, sl], scalar=-1.0,
                        in1=xc16[0][:, sl], op0=OP.mult, op1=OP.mult)
                    nc.sync.dma_start(u_scr[:, sl], U[:, sl])
            # Cm2 replication via gpsimd SWDGE (HC consumes it later, the
            # SWDGE latency is hidden); Bm2 was filled per-half above
            sap = cm_scr[:]
            for k in range(GK):
                nc.gpsimd.dma_start(Cm2[:, k * L:(k + 1) * L], bass.AP(
                    tensor=sap.tensor, offset=sap.offset,
                    ap=[sap.ap[0], [0, 2], sap.ap[1]]))

            # ---- phase 2: selective scan, pair layout (q = 2n + j) ----
            # pair p covers channels d0=2p, d1=2p+1; partitions hold (n, j)
            # scan loop: 16 groups of 4 pairs. dtrep via PE one-hot matmul
            # into PSUM (DMA replication of dt measured as SBUF-port
            # contention). The 4 scans of a group merge into ONE scan over
            # [128, 4096]: zeroing dA's first column of pairs k>=1 makes
            # the recurrence reset exactly (h = 0*h_prev + dBx).
            LG = GK * L
            # taper both ends: small first groups let the scan start before
            # the full warmup; small last groups shorten the exit chain
            GROUPS = ([(0, 1), (1, 1), (2, 2)] +
                      [(4 + 4 * g, 4) for g in range(13)] +
                      [(56, 2), (58, 2), (60, 2), (62, 1), (63, 1)])
            with tc.tile_pool(name="psl", bufs=1, space="PSUM") as psl:
              with (
                tc.tile_pool(name="psr", bufs=3, space="PSUM") as psr,
                tc.tile_pool(name="sl", bufs=2) as slp,
              ):
                Y_ps = psl.tile([DH, L], F32, tag="Y")
                for (p0, gk) in GROUPS:
                    urep = slp.tile([DH, LG], BF16, tag="urep")
                    dAt = slp.tile([DH, LG], F32, tag="dA")
                    sap = u_scr[:]
                    for k in range(gk):
                        p = p0 + k
                        ksl = slice(k * L, (k + 1) * L)
                        dtrep = psr.tile([DH, L], F32, tag="dtrep")
                        selRp = selR[:, DH * p:DH * (p + 1)]
                        for sl in HLF:
                            nc.tensor.matmul(dtrep[:, sl], selRp, DT[:, sl])
                        nc.scalar.activation(dAt[:, ksl], dtrep[:], AF.Exp,
                                             scale=aposp[:, 0:1])
                        ueng = nc.sync if k % 2 == 0 else nc.scalar
                        ueng.dma_start(urep[:, ksl], bass.AP(
                            tensor=sap.tensor, offset=sap.offset + 2 * p * L,
                            ap=[[0, DS], [L, 2], [1, L]]))
                        if k:
                            nc.gpsimd.memset(dAt[:, k * L:k * L + 1], 0.0)
                    gsl = slice(0, gk * L)
                    # in-place: dBx overwrites urep, HC overwrites Ht
                    nc.vector.tensor_tensor(out=urep[:, gsl],
                                            in0=urep[:, gsl],
                                            in1=Bm2[:, gsl], op=OP.mult)
                    Ht = slp.tile([DH, LG], BF16, tag="H")
                    nc.vector.tensor_tensor_scan(
                        out=Ht[:, gsl], data0=dAt[:, gsl], data1=urep[:, gsl],
                        initial=0.0, op0=OP.mult, op1=OP.add)
                    HCt = Ht
                    nc.vector.tensor_tensor(out=HCt[:, gsl], in0=Ht[:, gsl],
                                            in1=Cm2[:, gsl], op=OP.mult)
                    for k in range(gk):
                        p = p0 + k
                        selp = selE[:, DH * p:DH * (p + 1)]
                        for sl in HLF:
                            nc.tensor.matmul(
                                Y_ps[:, sl], selp,
                                HCt[:, k * L + sl.start:k * L + sl.stop],
                                start=(p == 0), stop=(p == DH // 2 - 1))

          # ---- tail: gate, out-proj, pool (pipelined by t-half) ----
              y2 = wp.tile([DH, L], F32)
              y3 = wp.tile([DH, L], F32)
              trash = wp.tile([DM, L], F32)
              pooled_h = wp.tile([DM, 2], F32)
              pooled = wp.tile([DM, 1], F32)
              with tc.tile_pool(name="ps2", bufs=1, space="PSUM") as ps2:
                  out_ps = ps2.tile([DM, L], F32, tag="o")
                  for hi, sl in enumerate(HLF):
                      nc.vector.scalar_tensor_tensor(
                          out=y2[:, sl], in0=xc16[0][:, sl], scalar=dskip[:],
                          in1=Y_ps[:, sl], op0=OP.mult, op1=OP.add)
                      nc.vector.tensor_tensor(out=y3[:, sl], in0=y2[:, sl],
                                              in1=zsig[:, sl], op=OP.mult)
                      nc.tensor.matmul(out_ps[:, sl], woutT[:, :], y3[:, sl])
                      nc.scalar.activation(
                          trash[:, sl], out_ps[:, sl], AF.Identity,
                          scale=1.0 / L, accum_out=pooled_h[:, hi:hi + 1])
                  nc.vector.tensor_tensor(
                      out=pooled[:], in0=pooled_h[:, 0:1],
                      in1=pooled_h[:, 1:2], op=OP.add)
                  nc.sync.dma_start(pooled_d[:], pooled[:])

    nc.compile()
    return nc


def _core_inputs(inputs, b, half):
    f32 = np.float32
    bf16 = ml_dtypes.bfloat16
    x = np.asarray(inputs["x"], f32)
    Wp = np.asarray(inputs["Wp"], f32)
    bp = np.asarray(inputs["bp"], f32)
    W_in = np.asarray(inputs["W_in"], f32)
    conv_w = np.asarray(inputs["conv_w"], f32)
    conv_b = np.asarray(inputs["conv_b"], f32)
    W_x = np.asarray(inputs["W_x"], f32)
    W_dt = np.asarray(inputs["W_dt"], f32)
    b_dt = np.asarray(inputs["b_dt"], f32)
    A_log = np.asarray(inputs["A_log"], f32)
    Dskip = np.asarray(inputs["Dskip"], f32)
    W_out = np.asarray(inputs["W_out"], f32)

    own = slice(half * DH, half * DH + DH)
    other = slice(DH, 2 * DH) if half == 0 else slice(0, DH)
    return {
        "xt": np.ascontiguousarray(x[b]),
        "wpT": np.ascontiguousarray(Wp.T),
        "bp": np.ascontiguousarray(bp[:, None]),
        "wiT": np.concatenate(
            [W_in[0:DI][own].T, W_in[0:DI][other].T,
             W_in[DI:2 * DI][own].T], axis=1),
        "convw": np.concatenate([conv_w[own], conv_w[other]], axis=1),
        "convb": np.stack([conv_b[own], conv_b[other]], axis=1),
        "wxT": np.concatenate([W_x.T[own], W_x.T[other]], axis=1),
        "wdtT": np.ascontiguousarray(W_dt[own].T),
        "bdt": np.ascontiguousarray(b_dt[own][:, None]),
        "alogp": _alog_pairs(A_log[own]),
        "dskip": np.ascontiguousarray(Dskip[own][:, None]),
        "woutT": np.ascontiguousarray(W_out[:, own].T),
        "selE": _selE(),
        "selR": _selR(),
    }


def _alog_pairs(alog_own):
    # alogp[q, p] = A_log[own][2p + q%2, q//2]
    out = np.empty((DH, DS), np.float32)
    q = np.arange(DH)
    for p in range(DS):
        out[:, p] = alog_own[2 * p + (q % 2), q // 2]
    return out


_sel_cache = {}


def _selE():
    if "v" not in _sel_cache:
        sel = np.zeros((DH, DS * DH), np.float32)
        q = np.arange(DH)
        for p in range(DS):
            sel[q, DH * p + 2 * p + (q % 2)] = 1.0
        _sel_cache["v"] = sel.astype(ml_dtypes.bfloat16)
    return _sel_cache["v"]


def _selR():
    if "r" not in _sel_cache:
        sel = np.zeros((DH, DS * DH), np.float32)
        q = np.arange(DH)
        for p in range(DS):
            sel[2 * p + (q % 2), DH * p + q] = 1.0
        _sel_cache["r"] = sel.astype(ml_dtypes.bfloat16)
    return _sel_cache["r"]


def kernel(**inputs) -> np.ndarray:
    if "nc" not in _cache:
        _cache["nc"] = _build()
    nc = _cache["nc"]

    in_maps = [_core_inputs(inputs, c // 2, c % 2) for c in range(8)]
    res = run_bass_kernel_spmd(nc, in_maps, core_ids=list(range(8)))

    pooled = np.zeros((B, DM), np.float32)
    for c in range(8):
        pooled[c // 2] += res.results[c]["pooled"][:, 0]

    # classifier head (host: BatchNorm couples all batches; ~300 flops)
    f32 = np.float32
    W1 = np.asarray(inputs["W1"], f32)
    b1 = np.asarray(inputs["b1"], f32)
    gamma = np.asarray(inputs["gamma"], f32)
    beta = np.asarray(inputs["beta"], f32)
    W2 = np.asarray(inputs["W2"], f32)
    b2 = np.asarray(inputs["b2"], f32)
    h1 = pooled @ W1.T + b1
    mu = h1.mean(axis=0)
    var = h1.var(axis=0)
    h1 = (h1 - mu) / np.sqrt(var + EPS) * gamma + beta
    h1 = np.maximum(h1, 0.0)
    return (h1 @ W2.T + b2).astype(np.float32)

